# revision 27
# baseline (speedup 1.0000x reference)
"""Trainium Bass kernel for nn_AdaptiveMoodCoherenceHysteresis.

Math (after simplification of the reference):
  triad = diff + diff^T == 0  =>  triad_mag diagonal == 1, so
  plv[b,f]  = |mean_t exp(i*phases[b,f,t])|
  coh[b]    = clip(mean_f sqrt(plv[b,f] + 1e-12), 0, 1)
  v         = coh - prev_coh
  alpha     = prev_alpha + BETA*(AMIN + (AMAX-AMIN)*sigmoid(8|v|-1.5) - prev_alpha)
  out       = prev_coh + alpha*v          (== alpha*coh + (1-alpha)*prev_coh)

Sharding: pure data parallel, one batch element per NeuronCore (B=8, 8 cores).

Per-core kernel ([64, 2048] f32 shard, viewed as [128, 1024], p = h*64+f):
  DVE : y = x/(2pi); k = rint(y) (magic const); d = y-k in [-.5,.5]; a = |d|
  ACT : sin(2pi*d) and cos = sin(pi/2 - 2pi*|d|), each with accum_out giving
        per-partition sums directly (no big reductions)
  PE  : per-chunk fold matmuls (0/1 matrix) accumulate transpose+h-fold into
        one PSUM row psC[0, 0:64]=cos_f, [0,64:128]=sin_f - fully overlapped
        with the streaming front
  DVE : plv2 = cos^2+sin^2; quarter-root plv2^(1/4) via bit-trick seed +
        1 Newton iter; fused row-sum
  ACT : whole hysteresis scalar chain via activation affine (AP scale/bias)
"""
import numpy as np
import concourse.bass as bass
import concourse.mybir as mybir
from concourse.bass_utils import run_bass_kernel_spmd

F, T = 64, 2048
P = 128                      # partitions: (h, f), h in {0,1} halves of T
W = T // 2                   # free dim per partition = 1024
N_CORES = 8

ALPHA_MIN = 0.08
ALPHA_MAX = 0.45
BETA = 0.12
SIG_SLOPE = 8.0
SIG_OFFSET = 1.5

TWO_PI = 2.0 * np.pi
INV_2PI = 1.0 / TWO_PI
MAGIC = 1.5 * 2.0 ** 23      # rint(x) = (x + MAGIC) - MAGIC for |x| < 2^22
K4 = 1331186000              # seed bits for x^(-1/4): K4 - (bits >> 2)
A = mybir.AluOpType
AF = mybir.ActivationFunctionType
F32 = mybir.dt.float32
I32 = mybir.dt.int32


def build(nchunk=2, repeat=1, debug=False, ablate=()):
    """Build the per-core Bass program. `repeat` re-runs the whole pipeline
    (differential wall-clock timing); output is from the last iteration."""
    C = W // nchunk
    nc = bass.Bass()
    ph_in = nc.declare_dram_parameter("phases", [F, T], F32, isOutput=False)
    # foldm[:, 0:64] = fold matrix; foldm[0, 64:66] = (prev_coh, prev_alpha)
    fm_in = nc.declare_dram_parameter("foldm", [P, F + 2], F32, isOutput=False)
    out_d = nc.declare_dram_parameter("out", [1, 1], F32, isOutput=True)
    if debug:
        dbg_acc = nc.declare_dram_parameter("dbg_acc", [P, 2 * nchunk], F32,
                                            isOutput=True)
        dbg_plv2 = nc.declare_dram_parameter("dbg_plv2", [1, 64], F32,
                                             isOutput=True)
        dbg_ut = nc.declare_dram_parameter("dbg_ut", [1, 64], F32,
                                           isOutput=True)
        dbg_sc = nc.declare_dram_parameter("dbg_sc", [1, 16], F32,
                                           isOutput=True)

    # [2, 64, 1024] view with (h, f, t) iteration order == [128, 1024] p=(h f)
    ph = ph_in[:].rearrange("f (h t) -> f h t", h=2).rearrange("f h t -> h f t")

    from contextlib import ExitStack
    with ExitStack() as stack:
        def sb(name, shape, dtype=F32):
            return stack.enter_context(nc.sbuf_tensor(name, shape, dtype))

        x0, x1 = sb("x0", [P, C]), sb("x1", [P, C])
        x2, x3 = sb("x2", [P, C]), sb("x3", [P, C])
        yt, kt = sb("yt", [P, C]), sb("kt", [P, C])
        d0, d1 = sb("d0", [P, C]), sb("d1", [P, C])
        a0, a1 = sb("a0", [P, C]), sb("a1", [P, C])
        scr = sb("scr", [P, C])
        acc = sb("acc", [P, 2 * nchunk])
        foldm = sb("foldm_sb", [P, F + 2])
        row = sb("row", [1, 128])
        sq = sb("sq", [1, 128])
        plv2 = sb("plv2", [1, 64])
        nt1 = sb("nt1", [1, 64])
        nt2 = sb("nt2", [1, 64])
        nt3 = sb("nt3", [1, 64])
        sc = sb("sc", [1, 16])    # small scalars, one per col
        halfpi = sb("halfpi", [P, 1])
        btanh = sb("btanh", [1, 1])
        psCos = stack.enter_context(nc.psum_tensor("psCos", [1, 64], F32))
        psSin = stack.enter_context(nc.psum_tensor("psSin", [1, 64], F32))
        ch_sem0 = stack.enter_context(nc.semaphore("ch_sem0"))
        ch_sem1 = stack.enter_context(nc.semaphore("ch_sem1"))
        fm_sem = stack.enter_context(nc.semaphore("fm_sem"))
        g_sem = stack.enter_context(nc.semaphore("g_sem"))
        v_sem = stack.enter_context(nc.semaphore("v_sem"))
        act_sem = stack.enter_context(nc.semaphore("act_sem"))
        pe_sem = stack.enter_context(nc.semaphore("pe_sem"))
        block = stack.enter_context(nc.Block())
        xb = [x0, x1, x2, x3]
        db = [d0, d1]
        ab = [a0, a1]
        ch_sem2 = stack.enter_context(nc.semaphore("ch_sem2"))
        ch_sem3 = stack.enter_context(nc.semaphore("ch_sem3"))
        ch_sems = [ch_sem0, ch_sem1, ch_sem2, ch_sem3]
        prev_t = foldm[0:1, F:F + 2]     # (prev_coh, prev_alpha) on part. 0
        # sc columns
        S_, V_, AV, TH, T2, ALP, NPC, OUT = range(8)

        # per-iteration semaphore deltas
        V_PER = nchunk + 1     # chunks, S
        ACT_PER = nchunk + 1   # chunk cos's, tail chain
        PE_PER = 1

        @block.sync
        def _(sync):
            sync.dma_start(foldm[:], fm_in[:]).then_inc(fm_sem, 16)
            for r in range(repeat):
                actb = r * ACT_PER
                for i in range(nchunk):
                    g = r * nchunk + i
                    if g >= 4:
                        # buffer reuse: chunk g-4's DVE must be done
                        pr, pi = divmod(g - 4, nchunk)
                        sync.wait_ge(v_sem, pr * V_PER + pi + 1)
                    sync.dma_start(
                        xb[g % 4][:], ph[:, :, i * C:(i + 1) * C]
                    ).then_inc(ch_sems[g % 4], 16)
                sync.wait_ge(act_sem, actb + nchunk + 1)
                sync.dma_start(out_d[:], sc[0:1, OUT:OUT + 1]).then_inc(g_sem, 16)
                if debug:
                    sync.dma_start(dbg_acc[:], acc[:]).then_inc(g_sem, 16)
                    sync.dma_start(dbg_plv2[:], plv2[:]).then_inc(g_sem, 16)
                    sync.dma_start(dbg_ut[:], nt3[:]).then_inc(g_sem, 16)
                    sync.dma_start(dbg_sc[:], sc[:]).then_inc(g_sem, 16)

        @block.vector
        def _(vector):
            vector.memset(halfpi[:], float(np.pi / 2))
            vector.memset(btanh[:], float(-SIG_OFFSET / 2.0))
            # prev-derived scalars for the ACT tail (constants per NEFF run)
            c2 = BETA * (ALPHA_MAX - ALPHA_MIN)
            vector.wait_ge(fm_sem, 16)
            vector.tensor_scalar_mul(sc[0:1, NPC:NPC + 1],
                                     prev_t[0:1, 0:1], -1.0)
            vector.tensor_scalar(sc[0:1, T2:T2 + 1], prev_t[0:1, 1:2],
                                 1.0 - BETA, BETA * ALPHA_MIN + 0.5 * c2,
                                 A.mult, A.add)
            for r in range(repeat):
                vb = r * V_PER
                peb = r * PE_PER
                for i in range(nchunk):
                    g = r * nchunk + i
                    vector.wait_ge(ch_sems[g % 4], 16 * (g // 4 + 1))
                    x = xb[g % 4]
                    vector.tensor_scalar_mul(yt[:], x[:], INV_2PI)
                    vector.tensor_scalar(kt[:], yt[:], MAGIC, MAGIC, A.add,
                                         A.subtract)
                    vector.tensor_sub(db[g % 2][:], yt[:], kt[:])
                    vector.scalar_tensor_tensor(
                        ab[g % 2][:], db[g % 2][:], -1.0, db[g % 2][:],
                        A.mult, A.max,
                    ).then_inc(v_sem, 1)
                # tail: psC = [cos_f | sin_f] already folded by PE
                vector.wait_ge(pe_sem, peb + 1)
                vector.tensor_copy(row[0:1, 0:64], psCos[:])
                vector.tensor_copy(row[0:1, 64:128], psSin[:])
                vector.tensor_mul(sq[:], row[:], row[:])
                vector.tensor_add(plv2[0:1, 0:64], sq[0:1, 0:64],
                                  sq[0:1, 64:128])
                # q = plv2^(-1/4): bit-trick seed + 1 Newton iter
                vector.tensor_scalar(nt1[:].bitcast(I32), plv2[:].bitcast(I32),
                                     2, None, A.arith_shift_right)
                vector.tensor_scalar(nt1[:].bitcast(I32), nt1[:].bitcast(I32),
                                     -1, K4, A.mult, A.add)   # q0
                newton_iters = 0 if "newton" in ablate else 1
                for _ in range(newton_iters):
                    vector.tensor_mul(nt2[:], nt1[:], nt1[:])       # q^2
                    vector.tensor_mul(nt2[:], nt2[:], nt2[:])       # q^4
                    vector.tensor_mul(nt2[:], nt2[:], plv2[:])      # x*q^4
                    vector.tensor_scalar(nt2[:], nt2[:], -0.25, 1.25,
                                         A.mult, A.add)
                    vector.tensor_mul(nt1[:], nt1[:], nt2[:])       # q'
                # u = plv2 * q^3 (= plv2^(1/4)), fused row-sum -> S
                vector.tensor_mul(nt2[:], nt1[:], nt1[:])
                vector.tensor_mul(nt3[:], nt2[:], nt1[:])
                vector.scalar_tensor_tensor(
                    nt3[:], nt3[:], 1.0, plv2[:], A.mult, A.mult,
                    accum_out=sc[0:1, S_:S_ + 1],
                ).then_inc(v_sem, 1)  # -> vb + nchunk + 1

        @block.tensor
        def _(tensor):
            tensor.wait_ge(fm_sem, 16)
            for r in range(repeat):
                actb = r * ACT_PER
                # psC[0,0:64] += fold(cos chunk i); [0,64:128] += fold(sin)
                for i in range(nchunk):
                    tensor.wait_ge(act_sem, actb + i + 1)
                    tensor.matmul(psCos[:], acc[:, nchunk + i:nchunk + i + 1],
                                  foldm[:, 0:F],
                                  start=(i == 0), stop=(i == nchunk - 1))
                    mm = tensor.matmul(psSin[:], acc[:, i:i + 1],
                                       foldm[:, 0:F],
                                       start=(i == 0), stop=(i == nchunk - 1))
                    if i == nchunk - 1:
                        mm.then_inc(pe_sem, 1)

        @block.scalar
        def _(scalar):
            # touch the Sin table set before any data waits so the one-time
            # ACT_TABLE_LOAD overlaps the input DMA instead of serializing
            zp = nc.const_aps.aps[(F32, 0.0)]
            scalar.activation(scr[0:1, 0:1], zp[0:1, 0:1], AF.Sin,
                              bias=0.0, scale=1.0)
            for r in range(repeat):
                vb = r * V_PER
                for i in range(nchunk):
                    g = r * nchunk + i
                    scalar.wait_ge(v_sem, vb + i + 1)
                    if r > 0 and i == 0:
                        # previous iteration's PE fold must have read acc
                        scalar.wait_ge(pe_sem, r)
                    scalar.activation(scr[:], db[g % 2][:], AF.Sin,
                                      bias=0.0, scale=TWO_PI,
                                      accum_out=acc[:, i:i + 1])
                    scalar.activation(scr[:], ab[g % 2][:], AF.Sin,
                                      bias=halfpi[:], scale=-TWO_PI,
                                      accum_out=acc[:, nchunk + i:nchunk + i + 1]
                                      ).then_inc(act_sem, 1)
                # hysteresis tail via activation affine with AP operands
                scalar.wait_ge(v_sem, vb + nchunk + 1)
                kcoh = float(1.0 / (F * np.sqrt(T)))
                scalar.activation(sc[0:1, V_:V_ + 1], sc[0:1, S_:S_ + 1],
                                  AF.Identity, bias=sc[0:1, NPC:NPC + 1],
                                  scale=kcoh)
                scalar.activation(sc[0:1, AV:AV + 1], sc[0:1, V_:V_ + 1],
                                  AF.Abs, bias=0.0, scale=1.0)
                scalar.activation(sc[0:1, TH:TH + 1], sc[0:1, AV:AV + 1],
                                  AF.Tanh, bias=btanh[:],
                                  scale=SIG_SLOPE / 2.0)
                c2 = BETA * (ALPHA_MAX - ALPHA_MIN)
                scalar.activation(sc[0:1, ALP:ALP + 1], sc[0:1, TH:TH + 1],
                                  AF.Identity, bias=sc[0:1, T2:T2 + 1],
                                  scale=0.5 * c2)
                scalar.activation(sc[0:1, OUT:OUT + 1], sc[0:1, V_:V_ + 1],
                                  AF.Identity, bias=prev_t[0:1, 0:1],
                                  scale=sc[0:1, ALP:ALP + 1]
                                  ).then_inc(act_sem, 1)

    return nc


_cache = {}


def _get_nc(nchunk=2, repeat=1):
    key = (nchunk, repeat)
    if key not in _cache:
        _cache[key] = build(nchunk=nchunk, repeat=repeat)
    return _cache[key]


def _fold_input(prev_coh_b, prev_alpha_b):
    fm = np.zeros((P, F + 2), dtype=np.float32)
    fm[:, :F] = np.tile(np.eye(F, dtype=np.float32), (2, 1))
    fm[0, F] = prev_coh_b
    fm[0, F + 1] = prev_alpha_b
    return fm


def kernel(phases, prev_coh, prev_alpha):
    phases = np.ascontiguousarray(np.asarray(phases, dtype=np.float32))
    prev_coh = np.asarray(prev_coh, dtype=np.float32)
    prev_alpha = np.asarray(prev_alpha, dtype=np.float32)
    B = phases.shape[0]
    assert B == N_CORES and phases.shape[1:] == (F, T)

    nc = _get_nc()
    in_maps = [
        {
            "phases": phases[b],
            "foldm": _fold_input(prev_coh[b], prev_alpha[b]),
        }
        for b in range(B)
    ]
    res = run_bass_kernel_spmd(nc, in_maps, core_ids=list(range(N_CORES))).results
    return np.array([res[b]["out"][0, 0] for b in range(B)], dtype=np.float32)


# revision 31
# speedup vs baseline: 1.1731x; 1.1731x over previous
"""Trainium Bass kernel for nn_AdaptiveMoodCoherenceHysteresis.

Math (after simplification of the reference):
  triad = diff + diff^T == 0  =>  triad_mag diagonal == 1, so
  plv[b,f]  = |mean_t exp(i*phases[b,f,t])|
  coh[b]    = clip(mean_f sqrt(plv[b,f] + 1e-12), 0, 1)
  v         = coh - prev_coh
  alpha     = prev_alpha + BETA*(AMIN + (AMAX-AMIN)*sigmoid(8|v|-1.5) - prev_alpha)
  out       = prev_coh + alpha*v          (== alpha*coh + (1-alpha)*prev_coh)

Sharding: pure data parallel, one batch element per NeuronCore (B=8, 8 cores).

Per-core kernel ([64, 2048] f32 shard, viewed as [128, 1024], p = h*64+f):
  DVE : y = x/(2pi); k = rint(y) (magic const); d = y-k in [-.5,.5]; a = |d|
  ACT : sin(2pi*d) and cos = sin(pi/2 - 2pi*|d|), each with accum_out giving
        per-partition sums directly (no big reductions)
  PE  : per-chunk fold matmuls (0/1 matrix) accumulate transpose+h-fold into
        one PSUM row psC[0, 0:64]=cos_f, [0,64:128]=sin_f - fully overlapped
        with the streaming front
  DVE : plv2 = cos^2+sin^2; quarter-root plv2^(1/4) via bit-trick seed +
        1 Newton iter; fused row-sum
  ACT : whole hysteresis scalar chain via activation affine (AP scale/bias)
"""
import numpy as np
import concourse.bass as bass
import concourse.mybir as mybir
from concourse.bass_utils import run_bass_kernel_spmd

F, T = 64, 2048
P = 128                      # partitions: (h, f), h in {0,1} halves of T
W = T // 2                   # free dim per partition = 1024
N_CORES = 8

ALPHA_MIN = 0.08
ALPHA_MAX = 0.45
BETA = 0.12
SIG_SLOPE = 8.0
SIG_OFFSET = 1.5

TWO_PI = 2.0 * np.pi
INV_2PI = 1.0 / TWO_PI
MAGIC = 1.5 * 2.0 ** 23      # rint(x) = (x + MAGIC) - MAGIC for |x| < 2^22
K4 = 1331186000              # seed bits for x^(-1/4): K4 - (bits >> 2)
A = mybir.AluOpType
AF = mybir.ActivationFunctionType
F32 = mybir.dt.float32
I32 = mybir.dt.int32


def build(nchunk=2, repeat=1, debug=False, ablate=()):
    """Build the per-core Bass program. `repeat` re-runs the whole pipeline
    (differential wall-clock timing); output is from the last iteration."""
    C = W // nchunk
    nc = bass.Bass()
    ph_in = nc.declare_dram_parameter("phases", [F, T], F32, isOutput=False)
    # foldm[:, 0:64] = fold matrix; foldm[0, 64:66] = (prev_coh, prev_alpha)
    fm_in = nc.declare_dram_parameter("foldm", [P, F + 2], F32, isOutput=False)
    out_d = nc.declare_dram_parameter("out", [1, 1], F32, isOutput=True)
    if debug:
        dbg_acc = nc.declare_dram_parameter("dbg_acc", [P, 2 * nchunk], F32,
                                            isOutput=True)
        dbg_plv2 = nc.declare_dram_parameter("dbg_plv2", [1, 64], F32,
                                             isOutput=True)
        dbg_ut = nc.declare_dram_parameter("dbg_ut", [1, 64], F32,
                                           isOutput=True)
        dbg_sc = nc.declare_dram_parameter("dbg_sc", [1, 16], F32,
                                           isOutput=True)

    # [2, 64, 1024] view with (h, f, t) iteration order == [128, 1024] p=(h f)
    ph = ph_in[:].rearrange("f (h t) -> f h t", h=2).rearrange("f h t -> h f t")

    from contextlib import ExitStack
    with ExitStack() as stack:
        def sb(name, shape, dtype=F32):
            return stack.enter_context(nc.sbuf_tensor(name, shape, dtype))

        x0, x1 = sb("x0", [P, C]), sb("x1", [P, C])
        x2, x3 = sb("x2", [P, C]), sb("x3", [P, C])
        yt, kt = sb("yt", [P, C]), sb("kt", [P, C])
        d0, d1 = sb("d0", [P, C]), sb("d1", [P, C])
        a0, a1 = sb("a0", [P, C]), sb("a1", [P, C])

        acc = sb("acc", [P, 2 * nchunk])
        foldm = sb("foldm_sb", [P, F + 2])
        row = sb("row", [1, 128])
        sq = sb("sq", [1, 128])
        plv2 = sb("plv2", [1, 64])
        nt1 = sb("nt1", [1, 64])
        nt2 = sb("nt2", [1, 64])
        nt3 = sb("nt3", [1, 64])
        sc = sb("sc", [1, 16])    # small scalars, one per col
        halfpi = sb("halfpi", [P, 1])
        btanh = sb("btanh", [1, 1])
        scr = stack.enter_context(nc.psum_tensor("scr", [P, C], F32))
        psCos = stack.enter_context(nc.psum_tensor("psCos", [1, 64], F32))
        psSin = stack.enter_context(nc.psum_tensor("psSin", [1, 64], F32))
        ch_sem0 = stack.enter_context(nc.semaphore("ch_sem0"))
        ch_sem1 = stack.enter_context(nc.semaphore("ch_sem1"))
        fm_sem = stack.enter_context(nc.semaphore("fm_sem"))
        g_sem = stack.enter_context(nc.semaphore("g_sem"))
        v_sem = stack.enter_context(nc.semaphore("v_sem"))
        act_sem = stack.enter_context(nc.semaphore("act_sem"))
        pe_sem = stack.enter_context(nc.semaphore("pe_sem"))
        block = stack.enter_context(nc.Block())
        xb = [x0, x1, x2, x3]
        db = [d0, d1]
        ab = [a0, a1]
        ch_sem2 = stack.enter_context(nc.semaphore("ch_sem2"))
        ch_sem3 = stack.enter_context(nc.semaphore("ch_sem3"))
        ch_sems = [ch_sem0, ch_sem1, ch_sem2, ch_sem3]
        prev_t = foldm[0:1, F:F + 2]     # (prev_coh, prev_alpha) on part. 0
        # sc columns
        S_, V_, AV, TH, T2, ALP, NPC, OUT = range(8)

        # per-iteration semaphore deltas
        V_PER = 2 * nchunk + 1   # per chunk: d-ready, a-ready; then S
        ACT_PER = nchunk + 1     # chunk cos's, tail chain
        PE_PER = 1

        @block.sync
        def _(sync):
            sync.dma_start(foldm[:], fm_in[:]).then_inc(fm_sem, 16)
            for r in range(repeat):
                actb = r * ACT_PER
                for i in range(nchunk):
                    g = r * nchunk + i
                    if g >= 4:
                        # buffer reuse: chunk g-4's DVE must be done
                        pr, pi = divmod(g - 4, nchunk)
                        sync.wait_ge(v_sem, pr * V_PER + 2 * pi + 2)
                    sync.dma_start(
                        xb[g % 4][:], ph[:, :, i * C:(i + 1) * C]
                    ).then_inc(ch_sems[g % 4], 16)
                sync.wait_ge(act_sem, actb + nchunk + 1)
                sync.dma_start(out_d[:], sc[0:1, OUT:OUT + 1]).then_inc(g_sem, 16)
                if debug:
                    sync.dma_start(dbg_acc[:], acc[:]).then_inc(g_sem, 16)
                    sync.dma_start(dbg_plv2[:], plv2[:]).then_inc(g_sem, 16)
                    sync.dma_start(dbg_ut[:], nt3[:]).then_inc(g_sem, 16)
                    sync.dma_start(dbg_sc[:], sc[:]).then_inc(g_sem, 16)

        @block.vector
        def _(vector):
            vector.memset(halfpi[:], float(np.pi / 2))
            vector.memset(btanh[:], float(-SIG_OFFSET / 2.0))
            # prev-derived scalars for the ACT tail (constants per NEFF run)
            c2 = BETA * (ALPHA_MAX - ALPHA_MIN)
            vector.wait_ge(fm_sem, 16)
            vector.tensor_scalar_mul(sc[0:1, NPC:NPC + 1],
                                     prev_t[0:1, 0:1], -1.0)
            vector.tensor_scalar(sc[0:1, T2:T2 + 1], prev_t[0:1, 1:2],
                                 1.0 - BETA, BETA * ALPHA_MIN + 0.5 * c2,
                                 A.mult, A.add)
            for r in range(repeat):
                vb = r * V_PER
                peb = r * PE_PER
                for i in range(nchunk):
                    g = r * nchunk + i
                    vector.wait_ge(ch_sems[g % 4], 16 * (g // 4 + 1))
                    x = xb[g % 4]
                    vector.tensor_scalar_mul(yt[:], x[:], INV_2PI)
                    vector.tensor_scalar(kt[:], yt[:], MAGIC, MAGIC, A.add,
                                         A.subtract)
                    vector.tensor_sub(db[g % 2][:], yt[:],
                                      kt[:]).then_inc(v_sem, 1)
                    vector.scalar_tensor_tensor(
                        ab[g % 2][:], db[g % 2][:], -1.0, db[g % 2][:],
                        A.mult, A.max,
                    ).then_inc(v_sem, 1)
                # tail: psC = [cos_f | sin_f] already folded by PE
                vector.wait_ge(pe_sem, peb + 1)
                vector.tensor_copy(row[0:1, 0:64], psCos[:])
                vector.tensor_copy(row[0:1, 64:128], psSin[:])
                vector.tensor_mul(sq[:], row[:], row[:])
                vector.tensor_add(plv2[0:1, 0:64], sq[0:1, 0:64],
                                  sq[0:1, 64:128])
                # q = plv2^(-1/4): bit-trick seed + 1 Newton iter
                vector.tensor_scalar(nt1[:].bitcast(I32), plv2[:].bitcast(I32),
                                     2, None, A.arith_shift_right)
                vector.tensor_scalar(nt1[:].bitcast(I32), nt1[:].bitcast(I32),
                                     -1, K4, A.mult, A.add)   # q0
                newton_iters = 0 if "newton" in ablate else 1
                for _ in range(newton_iters):
                    vector.tensor_mul(nt2[:], nt1[:], nt1[:])       # q^2
                    vector.tensor_mul(nt2[:], nt2[:], nt2[:])       # q^4
                    vector.tensor_mul(nt2[:], nt2[:], plv2[:])      # x*q^4
                    vector.tensor_scalar(nt2[:], nt2[:], -0.25, 1.25,
                                         A.mult, A.add)
                    vector.tensor_mul(nt1[:], nt1[:], nt2[:])       # q'
                # u = plv2 * q^3 (= plv2^(1/4)), fused row-sum -> S
                vector.tensor_mul(nt2[:], nt1[:], nt1[:])
                vector.tensor_mul(nt3[:], nt2[:], nt1[:])
                vector.scalar_tensor_tensor(
                    nt3[:], nt3[:], 1.0, plv2[:], A.mult, A.mult,
                    accum_out=sc[0:1, S_:S_ + 1],
                ).then_inc(v_sem, 1)  # -> vb + 2*nchunk + 1

        @block.tensor
        def _(tensor):
            tensor.wait_ge(fm_sem, 16)
            for r in range(repeat):
                actb = r * ACT_PER
                # psC[0,0:64] += fold(cos chunk i); [0,64:128] += fold(sin)
                for i in range(nchunk):
                    tensor.wait_ge(act_sem, actb + i + 1)
                    tensor.matmul(psCos[:], acc[:, nchunk + i:nchunk + i + 1],
                                  foldm[:, 0:F],
                                  start=(i == 0), stop=(i == nchunk - 1))
                    mm = tensor.matmul(psSin[:], acc[:, i:i + 1],
                                       foldm[:, 0:F],
                                       start=(i == 0), stop=(i == nchunk - 1))
                    if i == nchunk - 1:
                        mm.then_inc(pe_sem, 1)

        @block.scalar
        def _(scalar):
            # touch the Sin table set before any data waits so the one-time
            # ACT_TABLE_LOAD overlaps the input DMA instead of serializing
            zp = nc.const_aps.aps[(F32, 0.0)]
            scalar.activation(scr[0:1, 0:1], zp[0:1, 0:1], AF.Sin,
                              bias=0.0, scale=1.0)
            for r in range(repeat):
                vb = r * V_PER
                for i in range(nchunk):
                    g = r * nchunk + i
                    scalar.wait_ge(v_sem, vb + 2 * i + 1)
                    if r > 0 and i == 0:
                        # previous iteration's PE fold must have read acc
                        scalar.wait_ge(pe_sem, r)
                    scalar.activation(scr[:], db[g % 2][:], AF.Sin,
                                      bias=0.0, scale=TWO_PI,
                                      accum_out=acc[:, i:i + 1])
                    scalar.wait_ge(v_sem, vb + 2 * i + 2)
                    scalar.activation(scr[:], ab[g % 2][:], AF.Sin,
                                      bias=halfpi[:], scale=-TWO_PI,
                                      accum_out=acc[:, nchunk + i:nchunk + i + 1]
                                      ).then_inc(act_sem, 1)
                # hysteresis tail via activation affine with AP operands
                scalar.wait_ge(v_sem, vb + 2 * nchunk + 1)
                kcoh = float(1.0 / (F * np.sqrt(T)))
                scalar.activation(sc[0:1, V_:V_ + 1], sc[0:1, S_:S_ + 1],
                                  AF.Identity, bias=sc[0:1, NPC:NPC + 1],
                                  scale=kcoh)
                scalar.activation(sc[0:1, AV:AV + 1], sc[0:1, V_:V_ + 1],
                                  AF.Abs, bias=0.0, scale=1.0)
                scalar.activation(sc[0:1, TH:TH + 1], sc[0:1, AV:AV + 1],
                                  AF.Tanh, bias=btanh[:],
                                  scale=SIG_SLOPE / 2.0)
                c2 = BETA * (ALPHA_MAX - ALPHA_MIN)
                scalar.activation(sc[0:1, ALP:ALP + 1], sc[0:1, TH:TH + 1],
                                  AF.Identity, bias=sc[0:1, T2:T2 + 1],
                                  scale=0.5 * c2)
                scalar.activation(sc[0:1, OUT:OUT + 1], sc[0:1, V_:V_ + 1],
                                  AF.Identity, bias=prev_t[0:1, 0:1],
                                  scale=sc[0:1, ALP:ALP + 1]
                                  ).then_inc(act_sem, 1)

    return nc


_cache = {}


def _get_nc(nchunk=2, repeat=1):
    key = (nchunk, repeat)
    if key not in _cache:
        _cache[key] = build(nchunk=nchunk, repeat=repeat)
    return _cache[key]


def _fold_input(prev_coh_b, prev_alpha_b):
    fm = np.zeros((P, F + 2), dtype=np.float32)
    fm[:, :F] = np.tile(np.eye(F, dtype=np.float32), (2, 1))
    fm[0, F] = prev_coh_b
    fm[0, F + 1] = prev_alpha_b
    return fm


def kernel(phases, prev_coh, prev_alpha):
    phases = np.ascontiguousarray(np.asarray(phases, dtype=np.float32))
    prev_coh = np.asarray(prev_coh, dtype=np.float32)
    prev_alpha = np.asarray(prev_alpha, dtype=np.float32)
    B = phases.shape[0]
    assert B == N_CORES and phases.shape[1:] == (F, T)

    nc = _get_nc()
    in_maps = [
        {
            "phases": phases[b],
            "foldm": _fold_input(prev_coh[b], prev_alpha[b]),
        }
        for b in range(B)
    ]
    res = run_bass_kernel_spmd(nc, in_maps, core_ids=list(range(N_CORES))).results
    return np.array([res[b]["out"][0, 0] for b in range(B)], dtype=np.float32)
